# revision 9
# baseline (speedup 1.0000x reference)
"""Trainium2 Bass kernel for nn_MetaEmbedding_Classifier (retrieval_knn).

Strategy: shard the *weights* (not the batch) across 8 cores, since the
problem is memory-bound and data-parallel replication would make every core
read all ~41MB of weights.  Core k owns:
  - classes [125k, 125k+125) of centroids / W_hall / W_cos  (class shard)
  - rows    [256k, 256k+256) of W_sel                       (output-d shard)
  - columns [256k, 256k+256) of centroids                   (d shard, for
    the memory-feature matmul)
Per-core HBM traffic ~7MB instead of 41MB.  Two AllGather collectives tie
the cores together:
  AG#1: exp(hall-scores).T per core (+ local min-dist^2, sum-exp rows)
  AG#2: s.T chunks (s = x + infused) (+ local ssq rows)
All final per-row / per-class scaling is folded into
  logits[b,c] = 160/(sqrt(min2_b) + 10*sqrt(ssq_b)) * (s_b . Wcos_c/|Wcos_c|)
which follows from cosnorm(reach * s) with reach = 10/min_dist.

kernel(**inputs) takes FULL inputs and returns (logits, direct_feature,
infused_feature) exactly like the reference.
"""

import numpy as np

import concourse.bass as bass
import concourse.mybir as mybir
import concourse.tile as tile
from concourse import bacc
from concourse.bass_utils import run_bass_kernel_spmd
from concourse.masks import make_identity

FP = mybir.dt.float32
AX = mybir.AxisListType
ALU = mybir.AluOpType
ACT = mybir.ActivationFunctionType

B = 128        # batch
D = 2048       # feature dim
C = 1000       # classes
N_CORES = 8
CL = 125       # classes per core (unpadded)
CLP = 128      # padded classes per core
DL = 256       # d-chunk per core
NT = D // 128  # 16 d-tiles

NEG_BIG = -1.0e9

_BUILT = {}


def _trace(nc, tc, dbg=False):
    # ---- dram I/O handles (created by _build before TileContext) ----
    x_d = nc._io["x"]
    xck_d = nc._io["xck"]
    whall_d = nc._io["whall"]
    bhall_d = nc._io["bhall"]
    crows_d = nc._io["crows"]
    ccols_d = nc._io["ccols"]
    wsel_d = nc._io["wsel"]
    bsel_d = nc._io["bsel"]
    wcos_d = nc._io["wcos"]
    logits_d = nc._io["logits"]
    infused_d = nc._io["infused"]

    from contextlib import ExitStack
    ctx = ExitStack()
    pers = ctx.enter_context(tc.tile_pool(name="pers", bufs=1))
    scratch = ctx.enter_context(tc.tile_pool(name="scratch", bufs=2))
    pt = ctx.enter_context(tc.tile_pool(name="pt", bufs=3, space="PSUM"))
    pacc = ctx.enter_context(tc.tile_pool(name="pacc", bufs=4, space="PSUM"))
    dram = ctx.enter_context(tc.tile_pool(name="dram", bufs=1, space="DRAM"))

    ident = pers.tile([128, 128], FP, tag="ident")
    make_identity(nc, ident[:])
    ones_row = pers.tile([1, 128], FP, tag="ones_row")
    nc.gpsimd.memset(ones_row[:], 1.0)

    def pe_t(dst_ap, src_ap, tag="tp"):
        """PE transpose src [p, f] -> dst [f, p] via PSUM, then copy out."""
        p, f = src_ap.shape
        ps = pt.tile([f, p], FP, tag="tp")
        nc.tensor.transpose(ps[:], src_ap, ident[:p, :p])
        nc.any.tensor_copy(out=dst_ap, in_=ps[:])

    # ================= phase A: local stage-1 =================
    x_sb = pers.tile([B, D], FP, tag="x_sb")
    nc.sync.dma_start(x_sb[:], x_d[:])
    whall_sb = scratch.tile([CLP, D], FP, tag="w_big")
    nc.sync.dma_start(whall_sb[:], whall_d[:])
    crows_sb = pers.tile([CLP, D], FP, tag="crows_sb")
    nc.sync.dma_start(crows_sb[:], crows_d[:])
    bhall_sb = pers.tile([1, CLP], FP, tag="bhall_sb")
    nc.sync.dma_start(bhall_sb[:], bhall_d[:])
    xck_sb = pers.tile([B, DL], FP, tag="xck_sb")
    nc.sync.dma_start(xck_sb[:], xck_d[:])

    xT = pers.tile([128, NT, B], FP, tag="xT")
    for t in range(NT):
        pe_t(xT[:, t, :], x_sb[:, t * 128:(t + 1) * 128], tag="tp")

    ssq_x = pers.tile([B, 1], FP, tag="ssq_x")
    sq1 = scratch.tile([B, D], FP, tag="sq_big")
    nc.scalar.activation(sq1[:], x_sb[:], ACT.Square, accum_out=ssq_x[:])

    csq = pers.tile([CLP, 1], FP, tag="csq")
    sq2 = scratch.tile([B, D], FP, tag="sq_big")
    nc.scalar.activation(sq2[:], crows_sb[:], ACT.Square, accum_out=csq[:])

    whallT = pers.tile([128, NT, CLP], FP, tag="whallT")
    for t in range(NT):
        pe_t(whallT[:, t, :], whall_sb[:, t * 128:(t + 1) * 128], tag="tp")
    centT = pers.tile([128, NT, CLP], FP, tag="centT")
    for t in range(NT):
        pe_t(centT[:, t, :], crows_sb[:, t * 128:(t + 1) * 128], tag="tp")

    # -csq/2 as a [1, CLP] row; padded classes forced to NEG_BIG so the
    # running max over (x.c - csq/2) ignores them.
    csqnh = pers.tile([1, CLP], FP, tag="csqnh")
    ps_csq = pt.tile([1, CLP], FP, tag="tp")
    nc.tensor.transpose(ps_csq[:], csq[:], ident[:])
    nc.scalar.mul(csqnh[:], ps_csq[:], -0.5)
    nc.gpsimd.memset(csqnh[:, CL:CLP], NEG_BIG)

    # hall scores + exp (+ row-sum), min dist^2
    sh_ps = pacc.tile([B, CLP], FP, tag="acc")
    for t in range(NT):
        nc.tensor.matmul(sh_ps[:], xT[:, t, :], whallT[:, t, :],
                         start=(t == 0), stop=False)
    nc.tensor.matmul(sh_ps[:], ones_row[:], bhall_sb[:], start=False, stop=True)

    stats2 = pers.tile([B, 2], FP, tag="stats2")
    e_sb = pers.tile([B, CLP], FP, tag="e_sb")
    nc.scalar.activation(e_sb[:], sh_ps[:], ACT.Exp, accum_out=stats2[:, 1:2])

    xc_ps = pacc.tile([B, CLP], FP, tag="acc")
    for t in range(NT):
        nc.tensor.matmul(xc_ps[:], xT[:, t, :], centT[:, t, :],
                         start=(t == 0), stop=False)
    nc.tensor.matmul(xc_ps[:], ones_row[:], csqnh[:], start=False, stop=True)
    maxv = scratch.tile([B, 1], FP, tag="maxv")
    nc.vector.tensor_reduce(maxv[:], xc_ps[:], axis=AX.X, op=ALU.max)
    # min2 = ssq_x - 2*max(x.c - csq/2)
    nc.scalar.activation(stats2[:, 0:1], maxv[:], ACT.Identity,
                         scale=-2.0, bias=ssq_x[:])

    # pack [130, 128]: rows 0..127 = E.T, row 128 = min2, row 129 = sumexp
    cc1_in = dram.tile([CLP + 2, B], FP, tag="cc1_in")
    cc1_out = dram.tile([N_CORES * (CLP + 2), B], FP, tag="cc1_out",
                        addr_space="Shared")
    eT_sb = scratch.tile([CLP, B], FP, tag="eT_sb")
    pe_t(eT_sb[:], e_sb[:], tag="tp")
    nc.sync.dma_start(cc1_in[0:CLP, :], eT_sb[:])
    strow_sb = scratch.tile([2, B], FP, tag="strow_sb")
    pe_t(strow_sb[:], stats2[:], tag="tp")
    nc.sync.dma_start(cc1_in[CLP:CLP + 2, :], strow_sb[:])
    nc.gpsimd.collective_compute(
        "AllGather", ALU.bypass,
        replica_groups=[list(range(N_CORES))],
        ins=[cc1_in[:].opt()], outs=[cc1_out[:].opt()],
    )

    # ================= phase B: overlaps CC#1 =================
    wselT = pers.tile([128, NT, DL], FP, tag="wselT")
    for j in range(2):
        wsel_sb = scratch.tile([128, D], FP, tag="w_big")
        nc.sync.dma_start(wsel_sb[:], wsel_d[j * 128:(j + 1) * 128, :])
        for t in range(NT):
            pe_t(wselT[:, t, j * 128:(j + 1) * 128],
                 wsel_sb[:, t * 128:(t + 1) * 128], tag="tp")
    bsel_sb = pers.tile([1, DL], FP, tag="bsel_sb")
    nc.sync.dma_start(bsel_sb[:], bsel_d[:])

    wcos_sb = scratch.tile([CLP, D], FP, tag="w_big2")
    nc.sync.dma_start(wcos_sb[:], wcos_d[:])
    wsq = scratch.tile([CLP, 1], FP, tag="wsq")
    sq3 = scratch.tile([B, D], FP, tag="sq_big")
    nc.scalar.activation(sq3[:], wcos_sb[:], ACT.Square, accum_out=wsq[:])
    wnorm = scratch.tile([CLP, 1], FP, tag="wnorm")
    nc.scalar.sqrt(wnorm[:], wsq[:])
    wrecip = scratch.tile([CLP, 1], FP, tag="wrecip")
    nc.vector.reciprocal(wrecip[:], wnorm[:])
    wcos_n = scratch.tile([CLP, D], FP, tag="w_big3")
    nc.scalar.mul(wcos_n[:], wcos_sb[:], wrecip[:])
    wcosT = pers.tile([128, NT, CLP], FP, tag="wcosT")
    for t in range(NT):
        pe_t(wcosT[:, t, :], wcos_n[:, t * 128:(t + 1) * 128], tag="tp")

    ccols_sb = pers.tile([CLP, N_CORES, DL], FP, tag="ccols_sb")
    for r in range(N_CORES):
        nc.sync.dma_start(ccols_sb[:, r, :], ccols_d[r * CLP:(r + 1) * CLP, :])

    sel_ps = pacc.tile([B, DL], FP, tag="acc")
    for t in range(NT):
        nc.tensor.matmul(sel_ps[:], xT[:, t, :], wselT[:, t, :],
                         start=(t == 0), stop=False)
    nc.tensor.matmul(sel_ps[:], ones_row[:], bsel_sb[:], start=False, stop=True)
    sel_sb = pers.tile([B, DL], FP, tag="sel_sb")
    nc.scalar.activation(sel_sb[:], sel_ps[:], ACT.Tanh)

    # ================= phase C: after CC#1 =================
    et_sb = pers.tile([CLP, N_CORES, B], FP, tag="et_sb")
    for r in range(N_CORES):
        nc.sync.dma_start(et_sb[:, r, :],
                          cc1_out[r * (CLP + 2):r * (CLP + 2) + CLP, :])
    cc1_view = cc1_out[:].rearrange("(r q) w -> r q w", q=CLP + 2)
    stats_min = scratch.tile([N_CORES, B], FP, tag="stats_sb")
    nc.sync.dma_start(stats_min[:], cc1_view[:, CLP, :])
    stats_sum = scratch.tile([N_CORES, B], FP, tag="stats_sb")
    nc.sync.dma_start(stats_sum[:], cc1_view[:, CLP + 1, :])
    statsTm = scratch.tile([B, N_CORES], FP, tag="statsT")
    pe_t(statsTm[:], stats_min[:], tag="tp")
    statsTs = scratch.tile([B, N_CORES], FP, tag="statsT")
    pe_t(statsTs[:], stats_sum[:], tag="tp")
    gmin2 = pers.tile([B, 1], FP, tag="gmin2")
    nc.vector.tensor_reduce(gmin2[:], statsTm[:], axis=AX.X, op=ALU.min)
    gsum = scratch.tile([B, 1], FP, tag="gsum")
    nc.vector.tensor_reduce(gsum[:], statsTs[:], axis=AX.X, op=ALU.add)

    mf_ps = pacc.tile([B, DL], FP, tag="acc")
    for r in range(N_CORES):
        nc.tensor.matmul(mf_ps[:], et_sb[:, r, :], ccols_sb[:, r, :],
                         start=(r == 0), stop=(r == N_CORES - 1))
    rgsum = scratch.tile([B, 1], FP, tag="rgsum")
    nc.vector.reciprocal(rgsum[:], gsum[:])
    mfn_sb = scratch.tile([B, DL], FP, tag="mfn_sb")
    nc.scalar.mul(mfn_sb[:], mf_ps[:], rgsum[:])
    infused_sb = pers.tile([B, DL], FP, tag="infused_sb")
    nc.vector.tensor_mul(out=infused_sb[:], in0=mfn_sb[:], in1=sel_sb[:])
    nc.sync.dma_start(infused_d[:], infused_sb[:])

    s_sb = pers.tile([B, DL], FP, tag="s_sb")
    nc.vector.tensor_add(out=s_sb[:], in0=infused_sb[:], in1=xck_sb[:])
    ssq_p = scratch.tile([B, 1], FP, tag="ssq_p")
    sqs = scratch.tile([B, DL], FP, tag="sqs")
    nc.scalar.activation(sqs[:], s_sb[:], ACT.Square, accum_out=ssq_p[:])

    cc2_in = dram.tile([DL + 1, B], FP, tag="cc2_in")
    cc2_out = dram.tile([N_CORES * (DL + 1), B], FP, tag="cc2_out",
                        addr_space="Shared")
    for j in range(2):
        sT_sb = scratch.tile([128, B], FP, tag="sT_sb")
        pe_t(sT_sb[:], s_sb[:, j * 128:(j + 1) * 128], tag="tp")
        nc.sync.dma_start(cc2_in[j * 128:(j + 1) * 128, :], sT_sb[:])
    ssqrow_sb = scratch.tile([1, B], FP, tag="ssqrow_sb")
    pe_t(ssqrow_sb[:], ssq_p[:], tag="tp")
    nc.sync.dma_start(cc2_in[DL:DL + 1, :], ssqrow_sb[:])
    nc.gpsimd.collective_compute(
        "AllGather", ALU.bypass,
        replica_groups=[list(range(N_CORES))],
        ins=[cc2_in[:].opt()], outs=[cc2_out[:].opt()],
    )

    # ================= phase D: after CC#2 =================
    sT_full = pers.tile([128, NT, B], FP, tag="sT_full")
    for r in range(N_CORES):
        for j in range(2):
            base = r * (DL + 1) + j * 128
            nc.sync.dma_start(sT_full[:, r * 2 + j, :],
                              cc2_out[base:base + 128, :])
    ssq8_sb = scratch.tile([N_CORES, B], FP, tag="ssq8_sb")
    cc2_view = cc2_out[:].rearrange("(r q) w -> r q w", q=DL + 1)
    nc.sync.dma_start(ssq8_sb[:], cc2_view[:, DL, :])
    ssq8T = scratch.tile([B, N_CORES], FP, tag="ssq8T")
    pe_t(ssq8T[:], ssq8_sb[:], tag="tp")
    gssq = scratch.tile([B, 1], FP, tag="gssq")
    nc.vector.tensor_reduce(gssq[:], ssq8T[:], axis=AX.X, op=ALU.add)

    raw_ps = pacc.tile([B, CLP], FP, tag="acc")
    for t in range(NT):
        nc.tensor.matmul(raw_ps[:], sT_full[:, t, :], wcosT[:, t, :],
                         start=(t == 0), stop=(t == NT - 1))

    m_sb = scratch.tile([B, 1], FP, tag="m_sb")
    nc.scalar.sqrt(m_sb[:], gmin2[:])
    sn_sb = scratch.tile([B, 1], FP, tag="sn_sb")
    nc.scalar.sqrt(sn_sb[:], gssq[:])
    denom = scratch.tile([B, 1], FP, tag="denom")
    nc.scalar.activation(denom[:], sn_sb[:], ACT.Identity,
                         scale=10.0, bias=m_sb[:])
    rden = scratch.tile([B, 1], FP, tag="rden")
    nc.vector.reciprocal(rden[:], denom[:])
    kb = scratch.tile([B, 1], FP, tag="kb")
    nc.scalar.mul(kb[:], rden[:], 160.0)
    logits_sb = scratch.tile([B, CLP], FP, tag="logits_sb")
    nc.scalar.mul(logits_sb[:], raw_ps[:], kb[:])
    nc.sync.dma_start(logits_d[:], logits_sb[:])

    if dbg:
        def dump(name, ap):
            nc.sync.dma_start(nc._io[name][:], ap)
        dump("dbg_e", e_sb[:])
        dump("dbg_stats2", stats2[:])
        dump("dbg_gmin2", gmin2[:])
        dump("dbg_gsum", gsum[:])
        dump("dbg_mfn", mfn_sb[:])
        dump("dbg_sel", sel_sb[:])
        dump("dbg_s", s_sb[:])
        dump("dbg_ssqp", ssq_p[:])
        dump("dbg_gssq", gssq[:])
        dump("dbg_kb", kb[:])
        dump("dbg_xT0", xT[:, 0, :])
        dump("dbg_et0", et_sb[:, 0, :])
        dump("dbg_statsT", statsTs[:])
        dump("dbg_csqnh", csqnh[:])
        dump("dbg_whallT0", whallT[:, 0, :])
        nc.sync.dma_start(nc._io["dbg_cc1"][:], cc1_out[:])
        dump("dbg_stats_sb", stats_sum[:])

    ctx.close()


def _build(debug=False):
    key = "nc_dbg" if debug else "nc"
    if key in _BUILT:
        return _BUILT[key]
    nc = bacc.Bacc("TRN2", target_bir_lowering=False, num_devices=N_CORES)
    io = {}
    io["x"] = nc.dram_tensor("x", [B, D], FP, kind="ExternalInput")
    io["xck"] = nc.dram_tensor("xck", [B, DL], FP, kind="ExternalInput")
    io["whall"] = nc.dram_tensor("whall", [CLP, D], FP, kind="ExternalInput")
    io["bhall"] = nc.dram_tensor("bhall", [1, CLP], FP, kind="ExternalInput")
    io["crows"] = nc.dram_tensor("crows", [CLP, D], FP, kind="ExternalInput")
    io["ccols"] = nc.dram_tensor("ccols", [N_CORES * CLP, DL], FP,
                                 kind="ExternalInput")
    io["wsel"] = nc.dram_tensor("wsel", [DL, D], FP, kind="ExternalInput")
    io["bsel"] = nc.dram_tensor("bsel", [1, DL], FP, kind="ExternalInput")
    io["wcos"] = nc.dram_tensor("wcos", [CLP, D], FP, kind="ExternalInput")
    io["logits"] = nc.dram_tensor("logits", [B, CLP], FP, kind="ExternalOutput")
    io["infused"] = nc.dram_tensor("infused", [B, DL], FP,
                                   kind="ExternalOutput")
    if debug:
        for name, shape in [
            ("dbg_e", [B, CLP]), ("dbg_stats2", [B, 2]), ("dbg_gmin2", [B, 1]),
            ("dbg_gsum", [B, 1]), ("dbg_mfn", [B, DL]), ("dbg_sel", [B, DL]),
            ("dbg_s", [B, DL]), ("dbg_ssqp", [B, 1]), ("dbg_gssq", [B, 1]),
            ("dbg_kb", [B, 1]), ("dbg_xT0", [128, B]), ("dbg_et0", [CLP, B]),
            ("dbg_statsT", [B, N_CORES]), ("dbg_csqnh", [1, CLP]),
            ("dbg_whallT0", [128, CLP]),
            ("dbg_cc1", [N_CORES * (CLP + 2), B]),
            ("dbg_stats_sb", [N_CORES, B]),
        ]:
            io[name] = nc.dram_tensor(name, shape, FP, kind="ExternalOutput")
    nc._io = io
    with tile.TileContext(nc) as tc:
        _trace(nc, tc, dbg=debug)
    nc.compile()
    _BUILT[key] = nc
    return nc


def _build_null():
    """Same I/O signature, trivial body — calibrates dispatch overhead."""
    if "null" in _BUILT:
        return _BUILT["null"]
    nc = bacc.Bacc("TRN2", target_bir_lowering=False, num_devices=N_CORES)
    nc.dram_tensor("x", [B, D], FP, kind="ExternalInput")
    xck = nc.dram_tensor("xck", [B, DL], FP, kind="ExternalInput")
    nc.dram_tensor("whall", [CLP, D], FP, kind="ExternalInput")
    nc.dram_tensor("bhall", [1, CLP], FP, kind="ExternalInput")
    nc.dram_tensor("crows", [CLP, D], FP, kind="ExternalInput")
    nc.dram_tensor("ccols", [N_CORES * CLP, DL], FP, kind="ExternalInput")
    nc.dram_tensor("wsel", [DL, D], FP, kind="ExternalInput")
    nc.dram_tensor("bsel", [1, DL], FP, kind="ExternalInput")
    nc.dram_tensor("wcos", [CLP, D], FP, kind="ExternalInput")
    logits_d = nc.dram_tensor("logits", [B, CLP], FP, kind="ExternalOutput")
    infused_d = nc.dram_tensor("infused", [B, DL], FP, kind="ExternalOutput")
    with tile.TileContext(nc) as tc:
        with tc.tile_pool(name="sbuf", bufs=1) as pool:
            t = pool.tile([B, DL], FP)
            nc.sync.dma_start(t[:], xck[:])
            nc.sync.dma_start(infused_d[:], t[:])
            nc.sync.dma_start(logits_d[:], t[:, :CLP])
    nc.compile()
    _BUILT["null"] = nc
    return nc


def make_in_maps(inputs):
    f32 = np.float32
    x = np.ascontiguousarray(np.asarray(inputs["x"], f32))
    cent = np.asarray(inputs["centroids"], f32)
    W_hall = np.asarray(inputs["W_hall"], f32)
    b_hall = np.asarray(inputs["b_hall"], f32)
    W_sel = np.asarray(inputs["W_sel"], f32)
    b_sel = np.asarray(inputs["b_sel"], f32)
    W_cos = np.asarray(inputs["W_cos"], f32)

    in_maps = []
    for k in range(N_CORES):
        rows = slice(CL * k, CL * (k + 1))
        cols = slice(DL * k, DL * (k + 1))
        whall_p = np.zeros((CLP, D), f32)
        whall_p[:CL] = W_hall[rows]
        bhall_p = np.full((1, CLP), NEG_BIG, f32)
        bhall_p[0, :CL] = b_hall[rows]
        crows_p = np.zeros((CLP, D), f32)
        crows_p[:CL] = cent[rows]
        ccols_p = np.zeros((N_CORES, CLP, DL), f32)
        for r in range(N_CORES):
            ccols_p[r, :CL] = cent[CL * r:CL * (r + 1), cols]
        wcos_p = np.ones((CLP, D), f32)
        wcos_p[:CL] = W_cos[rows]
        in_maps.append({
            "x": x,
            "xck": np.ascontiguousarray(x[:, cols]),
            "whall": whall_p,
            "bhall": bhall_p,
            "crows": crows_p,
            "ccols": ccols_p.reshape(N_CORES * CLP, DL),
            "wsel": np.ascontiguousarray(W_sel[cols, :]),
            "bsel": np.ascontiguousarray(b_sel[cols]).reshape(1, DL),
            "wcos": wcos_p,
        })
    return in_maps


def assemble(results, x):
    logits = np.concatenate(
        [results[k]["logits"][:, :CL] for k in range(N_CORES)], axis=1
    )
    infused = np.concatenate(
        [results[k]["infused"] for k in range(N_CORES)], axis=1
    )
    return logits, np.asarray(x, np.float32).copy(), infused


def kernel(**inputs):
    nc = _build()
    in_maps = make_in_maps(inputs)
    res = run_bass_kernel_spmd(nc, in_maps, core_ids=list(range(N_CORES)))
    return assemble(res.results, inputs["x"])
